# revision 1
# baseline (speedup 1.0000x reference)
"""2-layer GCN on 8 TRN2 cores — v2.

v2 changes vs v1:
  - S (one-hot x norm, bf16) is built on HOST and DMA-loaded per chunk
    (kills the DVE is_equal expansion that dominated v1).
  - Phase D gathers from a device-built h2rep[50000, 64] f32 table (each row =
    h2[n] replicated), indexed by the SAME LO/HI src indices as phase A
    (idxA reused verbatim); extraction is a strided copy, no masking.
  - PSUM->SBUF evictions moved to the (idle) Activation engine.
"""
import math
import numpy as np
import ml_dtypes

import concourse.bass as bass
import concourse.bacc as bacc
import concourse.tile as tile
import concourse.mybir as mybir
from concourse import library_config

P = 128
REPW = 64         # h2rep row width (f32 -> 256B)
NBLK = 4          # dst blocks per chunk
MAX_DESC = 13000


def make_plan(edge_index, n_nodes, n_cores, lo_split=32768):
    src = np.concatenate([edge_index[0], np.arange(n_nodes)]).astype(np.int64)
    dst = np.concatenate([edge_index[1], np.arange(n_nodes)]).astype(np.int64)
    deg = np.bincount(dst, minlength=n_nodes).astype(np.float32)
    dinv = 1.0 / np.sqrt(np.maximum(deg, 1.0))
    norm = (dinv[src] * dinv[dst]).astype(np.float32)

    n_own = n_nodes // n_cores
    assert n_own * n_cores == n_nodes
    nb = math.ceil(n_own / P)
    n_pad = nb * P

    core = (dst // n_own).astype(np.int64)
    loc = dst - core * n_own
    blk = loc // P
    dloc = loc % P
    is_lo = src < lo_split

    order2 = np.lexsort((~is_lo, blk, core))
    src2, blk2, core2 = src[order2], blk[order2], core[order2]
    dloc2, norm2, islo2 = dloc[order2], norm[order2], is_lo[order2]

    counts = np.zeros((n_cores, nb, 2), np.int64)
    np.add.at(counts, (core2, blk2, (~islo2).astype(np.int64)), 1)
    kL = np.ceil(counts[:, :, 0] / P).astype(np.int64).max(axis=0)
    kH = np.ceil(counts[:, :, 1] / P).astype(np.int64).max(axis=0)

    chunks = []
    slot_ptr = 0
    for c0 in range(0, nb, NBLK):
        blocks = list(range(c0, min(c0 + NBLK, nb)))
        lo_start = slot_ptr
        lo_runs = {}
        for b in blocks:
            lo_runs[b] = (slot_ptr, slot_ptr + int(kL[b]))
            slot_ptr += int(kL[b])
        lo_end = slot_ptr
        hi_runs = {}
        for b in blocks:
            hi_runs[b] = (slot_ptr, slot_ptr + int(kH[b]))
            slot_ptr += int(kH[b])
        chunks.append(dict(blocks=blocks, lo=(lo_start, lo_end),
                           hi=(lo_end, slot_ptr), lo_runs=lo_runs,
                           hi_runs=hi_runs))
    tot = slot_ptr

    idx_a = np.zeros((n_cores, tot * P), np.int16)
    S_host = [np.zeros((P, tot * P), ml_dtypes.bfloat16) for _ in range(n_cores)]

    key = (core2 * nb + blk2) * 2 + (~islo2).astype(np.int64)
    run_start = np.searchsorted(key, np.arange(n_cores * nb * 2))

    for c in range(n_cores):
        for b in range(nb):
            ch = chunks[b // NBLK]
            for g, runs in ((0, ch["lo_runs"]), (1, ch["hi_runs"])):
                kidx = (c * nb + b) * 2 + g
                a = run_start[kidx]
                e = run_start[kidx + 1] if kidx + 1 < len(run_start) else len(src2)
                n = e - a
                if n == 0:
                    continue
                s0, s1 = runs[b]
                assert n <= (s1 - s0) * P
                fl = np.arange(n) + s0 * P
                v = src2[a:e] if g == 0 else src2[a:e] - lo_split
                idx_a[c, fl] = v.astype(np.int16)
                S_host[c][fl % P, (fl // P) * P + dloc2[a:e]] = norm2[a:e]

    def wrap(flat_i16, s0, s1):
        seg = flat_i16[s0 * P:s1 * P]
        return np.tile(seg.reshape(16, -1, order="F"), (8, 1))

    instrs_a = []
    for ch in chunks:
        if ch["lo"][1] > ch["lo"][0]:
            instrs_a.append(ch["lo"])
        if ch["hi"][1] > ch["hi"][0]:
            instrs_a.append(ch["hi"])
    split_a = []
    for s0, s1 in instrs_a:
        n = s1 - s0
        parts = math.ceil(n * P / MAX_DESC)
        step = math.ceil(n / parts)
        for a in range(s0, s1, step):
            split_a.append((a, min(a + step, s1)))

    idxA = [np.concatenate([wrap(idx_a[c], s0, s1) for (s0, s1) in split_a], axis=1)
            for c in range(n_cores)]

    return dict(
        n_own=n_own, nb=nb, n_pad=n_pad, tot=tot, chunks=chunks,
        instrs_a=split_a, lo_split=lo_split, idxA=idxA, S=S_host,
    )


def build_kernel(plan, n_nodes, n_cores, npass=1, phases="abcd"):
    nb, tot, n_pad = plan["nb"], plan["tot"], plan["n_pad"]
    n_own = plan["n_own"]
    lo_split = plan["lo_split"]
    assert n_nodes % 125 == 0
    RP, RK = 125, n_nodes // 125          # h2rep SBUF staging shape

    nc = bacc.Bacc("TRN2", target_bir_lowering=False, debug=False,
                   enable_asserts=False, num_devices=n_cores, num_swdge_queues=4)
    f32, bf16, i16 = mybir.dt.float32, mybir.dt.bfloat16, mybir.dt.int16

    xt = nc.dram_tensor("xt", [n_nodes, P], bf16, kind="ExternalInput").ap()
    idxA = nc.dram_tensor("idxA", [P, tot * 8], i16, kind="ExternalInput").ap()
    Sd = nc.dram_tensor("Sd", [P, tot * P], bf16, kind="ExternalInput").ap()
    w1 = nc.dram_tensor("w1", [P, P], bf16, kind="ExternalInput").ap()
    b1 = nc.dram_tensor("b1", [P, 1], f32, kind="ExternalInput").ap()
    w2 = nc.dram_tensor("w2", [P, 1], bf16, kind="ExternalInput").ap()
    b2 = nc.dram_tensor("b2", [P, 1], f32, kind="ExternalInput").ap()
    out = nc.dram_tensor("out", [n_pad, 1], f32, kind="ExternalOutput").ap()

    qn = [0]

    def next_q():
        qn[0] = (qn[0] + 1) % 4
        return qn[0]

    with tile.TileContext(nc) as tc:
        with (
            tc.tile_pool(name="const", bufs=1) as cpool,
            tc.tile_pool(name="dram", bufs=max(npass, 1), space="DRAM") as dpool,
        ):
            nc.gpsimd.load_library(library_config.mlp)
            idxA_t = cpool.tile([P, tot * 8], i16)
            w1_t = cpool.tile([P, P], bf16)
            b1_t = cpool.tile([P, 1], f32)
            w2_t = cpool.tile([P, 1], bf16)
            b2_t = cpool.tile([P, 1], f32)
            h2strip = cpool.tile([1, n_pad], f32)
            for t, d in ((idxA_t, idxA), (w1_t, w1), (b1_t, b1),
                         (w2_t, w2), (b2_t, b2)):
                nc.sync.dma_start(t[:], d[:])

            for _ps in range(npass):
                h2loc = dpool.tile([1, n_own], f32, name=f"h2loc{_ps}")
                h2all = dpool.tile([n_cores, n_own], f32, addr_space="Shared",
                                   name=f"h2all{_ps}")
                h2rep = dpool.tile([n_nodes, REPW], f32, name=f"h2rep{_ps}")

                # ---------------- phase A + B ----------------
                with (
                    tc.tile_pool(name="gbuf", bufs=2) as gpool,
                    tc.tile_pool(name="spool", bufs=2) as spool,
                    tc.tile_pool(name="small_ab", bufs=2) as smpool,
                    tc.tile_pool(name="agg_ps", bufs=3, space="PSUM") as agg_ps,
                    tc.tile_pool(name="o1_ps", bufs=2, space="PSUM") as o1_ps,
                    tc.tile_pool(name="h2_ps", bufs=2, space="PSUM") as h2_ps,
                ):
                    ia = 0
                    icol = 0
                    for ch in plan["chunks"]:
                        c0, c1 = ch["lo"][0], ch["hi"][1]
                        k = c1 - c0
                        G = gpool.tile([P, k * P], bf16, tag="G", name="G")
                        while ia < len(plan["instrs_a"]) and plan["instrs_a"][ia][0] < c1:
                            s0, s1 = plan["instrs_a"][ia]
                            n = s1 - s0
                            src_tab = (xt[:lo_split, :] if s0 < ch["hi"][0]
                                       else xt[lo_split:, :])
                            nc.gpsimd.dma_gather(
                                G[:, (s0 - c0) * P:(s1 - c0) * P].rearrange(
                                    "p (c d) -> p c d", d=P),
                                src_tab,
                                idxA_t[:, icol:icol + n * 8],
                                n * P, n * P, P,
                                single_packet=False, queue_num=next_q(),
                            )
                            icol += n * 8
                            ia += 1

                        S = spool.tile([P, k * P], bf16, tag="S", name="S")
                        nc.sync.dma_start(S[:], Sd[:, c0 * P:c1 * P])

                        for b in ch["blocks"]:
                            slots = (list(range(*ch["lo_runs"][b]))
                                     + list(range(*ch["hi_runs"][b])))
                            aggT = agg_ps.tile([P, P], f32, tag="agg", name="aggT")
                            for i, s in enumerate(slots):
                                sl = slice((s - c0) * P, (s - c0 + 1) * P)
                                nc.tensor.matmul(
                                    out=aggT[:], lhsT=G[:, sl], rhs=S[:, sl],
                                    start=(i == 0), stop=(i == len(slots) - 1))
                            aggT_sb = smpool.tile([P, P], bf16, tag="aggsb",
                                                  name="aggT_sb")
                            nc.scalar.activation(aggT_sb[:], aggT[:],
                                                 mybir.ActivationFunctionType.Identity)
                            o1 = o1_ps.tile([P, P], f32, tag="o1", name="o1")
                            nc.tensor.matmul(out=o1[:], lhsT=w1_t[:], rhs=aggT_sb[:],
                                             start=True, stop=True)
                            r1 = smpool.tile([P, P], bf16, tag="r1", name="r1")
                            nc.scalar.activation(r1[:], o1[:],
                                                 mybir.ActivationFunctionType.Relu,
                                                 bias=b1_t[:, 0:1])
                            h2p = h2_ps.tile([1, P], f32, tag="h2", name="h2p")
                            nc.tensor.matmul(out=h2p[:], lhsT=w2_t[:], rhs=r1[:],
                                             start=True, stop=True)
                            nc.vector.tensor_copy(
                                h2strip[0:1, b * P:(b + 1) * P], h2p[:])

                # ---------------- phase C ----------------
                if "c" not in phases:
                    nc.sync.dma_start(out[0:P, 0:1],
                                      h2strip[0:1, 0:P].rearrange("a b -> b a"))
                    continue
                nc.sync.dma_start(h2loc[:], h2strip[0:1, 0:n_own])
                nc.gpsimd.collective_compute(
                    "AllGather", mybir.AluOpType.bypass,
                    ins=[h2loc.opt()], outs=[h2all.opt()],
                    replica_groups=[list(range(n_cores))],
                )
                # build h2rep [n_nodes, REPW]
                with tc.tile_pool(name="rep", bufs=1) as rpool:
                    hin = rpool.tile([RP, RK], f32)
                    nc.sync.dma_start(
                        hin[:], h2all[:].rearrange("a b -> (a b)").rearrange(
                            "(p k) -> p k", p=RP))
                    hx = rpool.tile([RP, RK * REPW], f32)
                    nc.vector.tensor_copy(
                        hx[:].rearrange("p (k d) -> p k d", d=REPW),
                        hin[:].rearrange("p (k a) -> p k a", a=1).to_broadcast(
                            [RP, RK, REPW]))
                    nc.sync.dma_start(
                        h2rep[:].rearrange("(p k) d -> p (k d)", p=RP), hx[:])

                # ---------------- phase D ----------------
                if "d" not in phases:
                    nc.sync.dma_start(out[0:P, 0:1],
                                      h2strip[0:1, 0:P].rearrange("a b -> b a"))
                    continue
                with (
                    tc.tile_pool(name="repg", bufs=2) as repg,
                    tc.tile_pool(name="s2pool", bufs=2) as s2pool,
                    tc.tile_pool(name="smp2", bufs=2) as smp2,
                    tc.tile_pool(name="o2_ps", bufs=2, space="PSUM") as o2_ps,
                ):
                    ia = 0
                    icol = 0
                    for ch in plan["chunks"]:
                        c0, c1 = ch["lo"][0], ch["hi"][1]
                        k = c1 - c0
                        R = repg.tile([P, k * REPW], f32, tag="R", name="R")
                        while ia < len(plan["instrs_a"]) and plan["instrs_a"][ia][0] < c1:
                            s0, s1 = plan["instrs_a"][ia]
                            n = s1 - s0
                            rep_tab = (h2rep[:lo_split, :] if s0 < ch["hi"][0]
                                       else h2rep[lo_split:, :])
                            nc.gpsimd.dma_gather(
                                R[:, (s0 - c0) * REPW:(s1 - c0) * REPW].rearrange(
                                    "p (c d) -> p c d", d=REPW),
                                rep_tab,
                                idxA_t[:, icol:icol + n * 8],
                                n * P, n * P, REPW,
                                single_packet=False, queue_num=next_q(),
                            )
                            icol += n * 8
                            ia += 1

                        S = s2pool.tile([P, k * P], bf16, tag="S2", name="S2")
                        nc.sync.dma_start(S[:], Sd[:, c0 * P:c1 * P])
                        h2sb = smp2.tile([P, k], bf16, tag="h2s", name="h2sb")
                        nc.vector.tensor_copy(
                            h2sb[:],
                            R[:].rearrange("p (c d) -> p c d", d=REPW)[:, :, 0:1])

                        for b in ch["blocks"]:
                            slots = (list(range(*ch["lo_runs"][b]))
                                     + list(range(*ch["hi_runs"][b])))
                            o2 = o2_ps.tile([P, 1], f32, tag="o2", name="o2")
                            for i, s in enumerate(slots):
                                sl = slice((s - c0) * P, (s - c0 + 1) * P)
                                nc.tensor.matmul(
                                    out=o2[:], lhsT=S[:, sl],
                                    rhs=h2sb[:, (s - c0):(s - c0 + 1)],
                                    start=(i == 0), stop=(i == len(slots) - 1))
                            osb = smp2.tile([P, 1], f32, tag="osb", name="osb")
                            nc.vector.tensor_tensor(out=osb[:], in0=o2[:],
                                                    in1=b2_t[:, 0:1],
                                                    op=mybir.AluOpType.add)
                            nc.sync.dma_start(out[b * P:(b + 1) * P, 0:1], osb[:])

    nc.compile()
    return nc


def prepare_inputs(x, edge_index, W1, b1, W2, b2, n_cores, plan=None,
                   lo_split=32768):
    n_nodes = x.shape[0]
    if plan is None:
        plan = make_plan(np.asarray(edge_index), n_nodes, n_cores,
                         lo_split=lo_split)
    x_bf = np.asarray(x, np.float32).astype(ml_dtypes.bfloat16)
    w1_bf = np.asarray(W1, np.float32).astype(ml_dtypes.bfloat16)
    w2_bf = np.asarray(W2, np.float32).astype(ml_dtypes.bfloat16)
    b1c = np.asarray(b1, np.float32).reshape(P, 1)
    b2c = np.full((P, 1), np.asarray(b2, np.float32).reshape(-1)[0], np.float32)
    in_maps = []
    for c in range(n_cores):
        in_maps.append(dict(
            xt=x_bf, idxA=plan["idxA"][c], Sd=plan["S"][c],
            w1=w1_bf, b1=b1c, w2=w2_bf, b2=b2c,
        ))
    return plan, in_maps


def assemble_output(results, plan, n_nodes, n_cores):
    n_own = plan["n_own"]
    outs = [results[c]["out"][:n_own, :] for c in range(n_cores)]
    return np.concatenate(outs, axis=0)[:n_nodes].astype(np.float32)


# ======================================================================
# Self-contained kernel frontend (harness entry point)
# ======================================================================
from concourse import bass_utils as _bass_utils

N_NODES = 50000
N_CORES = 8
_kernel_cache = {}


def _plan_signature(plan):
    return (plan["tot"], plan["tot_d"], tuple(plan["instrs_a"]),
            tuple(plan["instrs_d"]),
            tuple((tuple(ch["blocks"]), ch["lo"], ch["hi"],
                   tuple(sorted(ch["lo_runs"].items())),
                   tuple(sorted(ch["hi_runs"].items())))
                  for ch in plan["chunks"]),
            tuple((tuple(ch["blocks"]), ch["lo"], ch["hi"],
                   tuple(sorted(ch["lo_runs"].items())),
                   tuple(sorted(ch["hi_runs"].items())))
                  for ch in plan["chunks_d"]))


def kernel(x, edge_index, W1, b1, W2, b2):
    """Full-input GCN forward on 8 NeuronCores; returns [N, 1] float32."""
    x = np.asarray(x)
    edge_index = np.asarray(edge_index)
    n_nodes = x.shape[0]
    plan = make_plan(edge_index, n_nodes, N_CORES)
    plan_d, in_maps = prepare_inputs(x, edge_index, W1, b1, W2, b2, N_CORES,
                                     plan=plan)
    sig = _plan_signature(plan)
    nc = _kernel_cache.get(sig)
    if nc is None:
        nc = build_kernel(plan, n_nodes, N_CORES)
        _kernel_cache[sig] = nc
    res = _bass_utils.run_bass_kernel_spmd(
        nc, in_maps, core_ids=list(range(N_CORES)))
    return assemble_output(res.results, plan, n_nodes, N_CORES)
